# revision 33
# baseline (speedup 1.0000x reference)
"""DTW loss kernel for Trainium2 (8 NeuronCores, pure batch data-parallel).

Problem: pred, targ [64, 384, 512] f32 -> mean over batch of DTW(cost_b),
cost_b[i,j] = ||pred[b,i]-targ[b,j]||_2.

Per core (8 batch items):
  1. Cost matrices via PE matmuls: -2*P^T@T accumulated with rank-1 terms
     (+|p_i|^2, +|t_j|^2) in PSUM, then sqrt on ACT, staged to DRAM (bf16).
  2. Wavefront DTW DP: the 384 columns are split into KC=16 chunks of L=24.
     Partition layout [(chunk k, item b) = 128 partitions, L cells]; chunk k
     lags chunk k-1 by LAG=3 wavefront steps, so all 16 chunks advance in
     parallel and the whole DP takes 384+45 steps of two small DVE ops
     ([128,24] tensor_tensor min for up/upleft, [128,25] tensor_tensor_scan
     for v[j] = min(m1[j], v[j-1]) + c[j]) instead of 384 steps of two
     [8,384] ops.  The chunk-boundary value crosses partitions via a tiny
     PE shift-matmul (shifted identity + BIG rank-1 for chunk 0) into PSUM,
     which the scan consumes directly as its per-partition `initial` AP;
     v[0] = min(M1[0]=BIG, initial) + c[0]=0 regenerates the boundary into
     V's column 0 where it doubles as next row's upleft.  No ACT/Pool work
     on the DP critical path; V is triple-buffered so the PE reads stay off
     it too.  The phase-1.5/mi=1,2 front work is spliced into the early DP
     steps as fine-grained per-engine units (DMA loads, Pool downcasts,
     DVE/ACT norms+copies, PE transposes), scheduled long before the DP
     consumes their cost rows - the slack matters for correctness, because
     DMA-to-compute sync only waits on one DMA queue semaphore.
"""

from contextlib import ExitStack

import numpy as np

import concourse.bacc as bacc
import concourse.mybir as mybir
import concourse.tile as tile
from concourse.bass_utils import run_bass_kernel_spmd
from concourse.masks import make_identity

B, T, D = 64, 384, 512
NCORES = 8
BPC = B // NCORES  # batches per core
F32 = mybir.dt.float32
BF16 = mybir.dt.bfloat16
BIG = 1.0e30
PP = 128  # partition tile
RB = T // PP  # 3 row blocks
KB = D // PP  # 4 contraction blocks
AF = mybir.ActivationFunctionType
ALU = mybir.AluOpType

# wavefront DP geometry
KC = 16          # column chunks
L = T // KC      # 24 cells per chunk
LAG = 3          # wavefront lag between adjacent chunks
STEPS = T + LAG * (KC - 1)       # 429
PADF = 48        # zero pad rows in front of cost matrix (>= LAG*(KC-1))
ROWS = PADF + T + PADF           # 480
G = 33           # steps per streamed cost tile; 13*33 = 429


def _dp_probe(ctx, tc, out, variant):
    """Timing-only probes for the DP inner loop (results are garbage)."""
    nc = tc.nc
    dp = ctx.enter_context(tc.tile_pool(name="dp", bufs=1))
    cstream = ctx.enter_context(tc.tile_pool(name="cstream", bufs=3))
    V = dp.tile([PP, 3, L + 1], F32, tag="V")
    M1 = dp.tile([PP, 4, L + 1], F32, tag="M1")
    nc.vector.memset(V, 1.0)
    nc.vector.memset(M1, 1.0)
    ct = dp.tile([PP, G, L + 1], BF16, tag="ct")
    nc.vector.memset(ct, 0.125)
    for s in range(STEPS):
        g = s % G
        bprev = (s - 1) % 3
        bcur = s % 3
        m1b = s % 4
        if variant in ("dptt", "dpnoact"):
            # dptt: chain through V so the ops serialize like the real DP
            ttout = V[:, bcur, 1:L + 1] if variant == "dptt" \
                else M1[:, m1b, 1:L + 1]
            nc.vector.tensor_tensor(
                out=ttout, in0=V[:, bprev, 1:L + 1],
                in1=V[:, bprev, 0:L], op=ALU.min)
        if variant in ("dpscan", "dpnoact"):
            d0 = M1[:, m1b, 0:L + 1] if variant == "dpnoact" \
                else V[:, bprev, 0:L + 1]
            nc.vector.tensor_tensor_scan(
                out=V[:, bcur, 0:L + 1], data0=d0,
                data1=ct[:, g, 0:L + 1], initial=BIG, op0=ALU.min, op1=ALU.add)
    nc.sync.dma_start(out=out[:, :],
                      in_=V[0:BPC, (STEPS - 1) % 3, L:L + 1])


def _kernel_body(ctx, tc, out, pred, targ, variant="full", repeats=1,
                 rep_barrier=False):
    for i in range(repeats):
        if rep_barrier and i:
            tc.strict_bb_all_engine_barrier()
        with ExitStack() as rep_ctx:
            _kernel_body_once(rep_ctx, tc, out, pred, targ, variant)


def _kernel_body_once(ctx, tc, out, pred, targ, variant="full"):
    nc = tc.nc
    if variant in ("dpscan", "dptt", "dpnoact"):
        _dp_probe(ctx, tc, out, variant)
        return
    do_front = variant in ("full", "front", "ss", "ssser", "ssscrub")
    do_dp = variant in ("full", "dp", "ss", "ssser", "ssscrub")
    serial_front = variant in ("full", "ssser")
    scrub = variant == "ssscrub"

    const = ctx.enter_context(tc.tile_pool(name="const", bufs=1))
    nat = ctx.enter_context(tc.tile_pool(name="nat", bufs=2))
    persist = ctx.enter_context(tc.tile_pool(name="persist", bufs=1))
    work = ctx.enter_context(tc.tile_pool(name="work", bufs=2))
    csb = ctx.enter_context(tc.tile_pool(name="csb", bufs=3))
    dp = ctx.enter_context(tc.tile_pool(name="dp", bufs=1))
    cstream = ctx.enter_context(tc.tile_pool(name="cstream", bufs=4))
    ptr = ctx.enter_context(tc.tile_pool(name="ptr", bufs=3, space="PSUM"))
    pacc = ctx.enter_context(tc.tile_pool(name="pacc", bufs=2, space="PSUM"))
    pvec = ctx.enter_context(tc.tile_pool(name="pvec", bufs=1, space="PSUM"))
    pbnd = ctx.enter_context(tc.tile_pool(name="pbnd", bufs=2, space="PSUM"))
    dram = ctx.enter_context(tc.tile_pool(name="dram", bufs=1, space="DRAM"))

    ident = const.tile([PP, PP], F32)
    make_identity(nc, ident)
    ones_row = const.tile([1, T], F32)
    nc.vector.memset(ones_row, 1.0)
    # shifted identity: shid[c, m] = 1 iff m = c + BPC  (partition shift +8)
    shid = const.tile([PP, PP], F32, tag="shid")
    nc.gpsimd.memset(shid, 0.0)
    nc.gpsimd.affine_select(
        out=shid, in_=shid, compare_op=ALU.not_equal, fill=1.0,
        base=BPC, pattern=[[-1, PP]], channel_multiplier=1)
    # rank-1 helpers to fill partitions 0..BPC of the boundary column with BIG
    bigrow = const.tile([1, PP], F32, tag="bigrow")
    nc.vector.memset(bigrow, 0.0)
    nc.vector.memset(bigrow[:, 0:BPC], BIG)
    one11 = const.tile([1, 1], F32, tag="one11")
    nc.vector.memset(one11, 1.0)

    # bf16 cost staging in DRAM, with PADF zero rows on both ends so the
    # wavefront's out-of-range rows read as zero cost.
    cost_dram = dram.tile([BPC, ROWS, T], BF16)
    ztile = const.tile([PP, T], BF16, tag="ztile")
    nc.vector.memset(ztile, 0.0)
    if scrub:
        # race-test mode: fill the whole staging buffer with poison first so
        # any read-before-write shows up on every run, not just the first.
        gtile = const.tile([PP, T], BF16, tag="gtile")
        nc.vector.memset(gtile, -1.0e37)
        for b in range(BPC):
            for r0 in range(0, ROWS, PP):
                n = min(PP, ROWS - r0)
                nc.sync.dma_start(out=cost_dram[b, r0:r0 + n, :],
                                  in_=gtile[0:n, :])
    for b in range(BPC if do_dp else 0):
        nc.sync.dma_start(out=cost_dram[b, 0:PADF, :], in_=ztile[0:PADF, :])
        nc.sync.dma_start(out=cost_dram[b, PADF + T:ROWS, :],
                          in_=ztile[0:PADF, :])

    identb = const.tile([PP, PP], BF16, tag="identb")
    nc.vector.tensor_copy(out=identb, in_=ident)
    natb = ctx.enter_context(tc.tile_pool(name="natb", bufs=2))

    def _norm_sq(src, ncol, rs, eng="dve"):
        # square with accum_out -> per-row-chunk column sums [128,1]
        for ri, r in enumerate(rs):
            sqd = work.tile([PP, D], BF16, tag="sqd")
            if eng == "dve":
                nc.vector.scalar_tensor_tensor(
                    out=sqd, in0=src[:, ri, :], scalar=1.0, in1=src[:, ri, :],
                    op0=ALU.mult, op1=ALU.mult,
                    accum_out=ncol[:, ri:ri + 1])
            else:
                nc.scalar.activation(
                    out=sqd, in_=src[:, ri, :], func=AF.Square,
                    accum_out=ncol[:, ri:ri + 1])

    def _norm_flip(ncol, dst, rs):
        # tiny identity-matmul flips each [128,1] to a [1,128] row of dst
        for ri, r in enumerate(rs):
            nps = pvec.tile([1, PP], F32, tag="nps")
            nc.tensor.matmul(nps, ncol[:, ri:ri + 1], ident)
            nc.scalar.activation(out=dst[:, r * PP:(r + 1) * PP], in_=nps,
                                 func=AF.Copy)

    pt2s, tts, pns, tns = [], [], [], []
    # phase 1: everything the mi=0 cost chunks need. P rows 128..384
    # (r=1,2) are deferred so the DP can start sooner.  Inputs are
    # downcast to bf16 (DVE, idle pre-DP) so the PE transposes run in the
    # bf16 tier and norms can run on the Pool engine.
    for b in range(BPC if do_front else 0):
        p_nat0 = nat.tile([PP, 1, D], F32, tag="p_nat0")
        t_nat = nat.tile([PP, RB, D], F32, tag="t_nat")
        nc.sync.dma_start(out=p_nat0[:, 0, :], in_=pred[b, 0:PP, :])
        for r in range(RB):
            nc.sync.dma_start(out=t_nat[:, r, :], in_=targ[b, r * PP:(r + 1) * PP, :])
        pbf0 = natb.tile([PP, 1, D], BF16, tag="pbf0")
        tbf = natb.tile([PP, RB, D], BF16, tag="tbf")
        nc.gpsimd.tensor_copy(out=pbf0[:, 0, :], in_=p_nat0[:, 0, :])
        for r in range(RB):
            eng = nc.gpsimd if r % 2 == 0 else nc.vector
            eng.tensor_copy(out=tbf[:, r, :], in_=t_nat[:, r, :])
        pnc = persist.tile([PP, RB], F32, tag=f"pnc_{b}")
        ncol = work.tile([PP, RB], F32, tag=f"ncol_{b}")
        _norm_sq(pbf0, pnc[:, 0:1], [0], "dve")
        _norm_sq(tbf, ncol, list(range(RB)), "dve")
        tn_sb = persist.tile([1, T], F32, tag=f"tn_{b}")
        _norm_flip(ncol, tn_sb, list(range(RB)))

        # pt2 = -2 * P^T  [d, i], tt = T^T [d, j], via bf16 PE transpose.
        # All KB k-chunks of one row block go into one PSUM tile so a
        # single strided copy moves them to SBUF.
        pt2 = persist.tile([PP, KB, T], BF16, tag=f"pt2_{b}")
        tt = persist.tile([PP, KB, T], BF16, tag=f"tt_{b}")
        for r in range(RB):
            ps4 = ptr.tile([PP, KB, PP], BF16, tag="tr")
            for k in range(KB):
                nc.tensor.transpose(ps4[:, k, :], tbf[:, r, k * PP:(k + 1) * PP],
                                    identb)
            if r % 2 == 0:
                nc.vector.tensor_copy(
                    out=tt[:, :, r * PP:(r + 1) * PP], in_=ps4)
            else:
                nc.scalar.activation(
                    out=tt[:, :, r * PP:(r + 1) * PP], in_=ps4, func=AF.Copy)
        ps4 = ptr.tile([PP, KB, PP], BF16, tag="tr")
        for k in range(KB):
            nc.tensor.transpose(ps4[:, k, :], pbf0[:, 0, k * PP:(k + 1) * PP],
                                identb)
        nc.scalar.activation(
            out=pt2[:, :, 0:PP], in_=ps4, func=AF.Copy, scale=-2.0)

        pt2s.append(pt2)
        tts.append(tt)
        pns.append(pnc)
        tns.append(tn_sb)

    def _cost_chunk(b, mi):
        pc = pacc.tile([PP, T], F32, tag="pc")
        for k in range(KB):
            nc.tensor.matmul(
                pc, pt2s[b][:, k, mi * PP:(mi + 1) * PP], tts[b][:, k, :],
                start=(k == 0), stop=False)
        nc.tensor.matmul(
            pc, ones_row[:, :PP], tns[b], start=False, stop=True)
        # sqrt(tn_j - 2G + pn_i): pn folded in as the per-partition bias.
        # No relu clamp: sq_dist = |p_i - t_j|^2 concentrates at ~2D +- ~90.
        cchunk = csb.tile([PP, T], BF16, tag="cchunk")
        nc.scalar.activation(out=cchunk, in_=pc, func=AF.Sqrt,
                             bias=pns[b][:, mi:mi + 1])
        nc.sync.dma_start(
            out=cost_dram[b, PADF + mi * PP:PADF + (mi + 1) * PP, :],
            in_=cchunk)

    # mi=0 chunks ASAP -- they gate the DP start
    for b in range(BPC if do_front else 0):
        _cost_chunk(b, 0)

    # phase 1.5 (off the DP-start critical path): reload P r=1,2 from DRAM,
    # finish pn and the remaining P transposes.  Serial fallback for the
    # front-only variant; the full kernel interleaves the same work into
    # the DP loop via front_sched units below.
    p12pool = ctx.enter_context(tc.tile_pool(name="p12pool", bufs=BPC))

    def _phase15(b):
        p_nat12 = p12pool.tile([PP, RB - 1, D], F32, tag="p_nat12")
        for r in range(1, RB):
            nc.sync.dma_start(out=p_nat12[:, r - 1, :],
                              in_=pred[b, r * PP:(r + 1) * PP, :])
        pb = persist.tile([PP, RB - 1, D], BF16, tag=f"p12b_{b}")
        for r in range(1, RB):
            nc.vector.tensor_copy(out=pb[:, r - 1, :], in_=p_nat12[:, r - 1, :])
        _norm_sq(pb, pns[b][:, 1:RB], list(range(1, RB)), "act")
        for r in range(1, RB):
            ps4 = ptr.tile([PP, KB, PP], BF16, tag="tr")
            for k in range(KB):
                nc.tensor.transpose(ps4[:, k, :],
                                    pb[:, r - 1, k * PP:(k + 1) * PP], identb)
            nc.scalar.activation(
                out=pt2s[b][:, :, r * PP:(r + 1) * PP], in_=ps4, func=AF.Copy,
                scale=-2.0)

    front_sched = {}

    if do_front and do_dp and not serial_front:
        # Remaining front work (P rows 128..384: loads, downcasts, norms,
        # transposes, and the mi=1,2 cost chunks) is spliced into the DP
        # loop as fine-grained units so each engine's queue stays shallow:
        # DMA loads, Pool downcasts+norms, PE transposes, ACT copies+sqrt.
        p12f, p12b = {}, {}

        def u_load(b):
            t = p12pool.tile([PP, RB - 1, D], F32, tag="p_nat12",
                             name=f"pn12_{b}")
            for r in range(1, RB):
                nc.sync.dma_start(out=t[:, r - 1, :],
                                  in_=pred[b, r * PP:(r + 1) * PP, :])
            p12f[b] = t

        def u_down(b, r):
            if b not in p12b:
                p12b[b] = persist.tile([PP, RB - 1, D], BF16,
                                       tag=f"p12b_{b}", name=f"p12b_{b}")
            nc.gpsimd.tensor_copy(out=p12b[b][:, r - 1, :],
                                  in_=p12f[b][:, r - 1, :])

        def u_norm(b, r):
            sqd = work.tile([PP, D], BF16, tag="sqd")
            nc.scalar.activation(
                out=sqd, in_=p12b[b][:, r - 1, :], func=AF.Square,
                accum_out=pns[b][:, r:r + 1])

        def u_tc(b, r):
            ps4 = ptr.tile([PP, KB, PP], BF16, tag="tr")
            for k in range(KB):
                nc.tensor.transpose(ps4[:, k, :],
                                    p12b[b][:, r - 1, k * PP:(k + 1) * PP],
                                    identb)
            nc.scalar.activation(out=pt2s[b][:, :, r * PP:(r + 1) * PP],
                                 in_=ps4, func=AF.Copy, scale=-2.0)

        units = []
        for b in range(BPC):
            units.append(lambda b=b: u_load(b))
        for b in range(BPC):
            units.append(lambda b=b: u_down(b, 1))
        for b in range(BPC):
            units.append(lambda b=b: u_norm(b, 1))
        for b in range(BPC):
            units.append(lambda b=b: u_tc(b, 1))
            units.append(lambda b=b: _cost_chunk(b, 1))
        for b in range(BPC):
            units.append(lambda b=b: u_down(b, 2))
            units.append(lambda b=b: u_norm(b, 2))
        for b in range(BPC):
            units.append(lambda b=b: u_tc(b, 2))
            units.append(lambda b=b: _cost_chunk(b, 2))
        for i, u in enumerate(units):
            front_sched.setdefault(1 + i // 2, []).append(u)
    elif do_front:
        for b in range(BPC):
            _phase15(b)
        for mi in range(1, RB):
            for b in range(BPC):
                _cost_chunk(b, mi)

    if not do_dp:
        # still produce an output so the NEFF has a defined result
        vdummy = dp.tile([BPC, 1], F32)
        nc.vector.memset(vdummy, 0.0)
        nc.sync.dma_start(out=out[:, :], in_=vdummy)
        return

    # ---------------- wavefront DTW DP ----------------
    # V, M1: [128 part = (chunk k)*8 + b, buf, L+1]; col 0 = boundary slot,
    # cols 1..L = cells.  The chunk-boundary value B_s (chunk k-1's last
    # cell, LAG steps ago, shifted +8 partitions by the PE) is fed to the
    # scan as its per-partition `initial` AP, read straight from PSUM:
    # v[0] = min(M1[0]=BIG, B_s) + c[0]=0 = B_s, which also maintains V's
    # column 0 as next row's upleft.  No ACT involvement in the DP at all.
    V = dp.tile([PP, 3, L + 1], F32, tag="V")
    M1 = dp.tile([PP, 2, L + 1], F32, tag="M1")
    nc.vector.memset(V, BIG)
    nc.vector.memset(V[0:BPC, :, 0:1], 0.0)   # DP corner (row -1, col -1) = 0
    nc.vector.memset(M1, BIG)                 # col 0 stays BIG forever

    bnds = []  # psum [128, 2] boundary tiles, one per step pair

    def _pe_shift(s):
        # boundary values for step s: B[p] = V[p-8, last cell] after
        # scan_{s-LAG} for p >= 8; BIG for p < 8 (chunk-0 left edge).
        if s % 2 == 0:
            bnd = pbnd.tile([PP, 2], F32, tag="bnd", name=f"bnd_{s}")
            bnds.append(bnd)
        col = bnds[s // 2][:, (s % 2):(s % 2) + 1]
        src = V[:, (s - LAG) % 3, L:L + 1]
        nc.tensor.matmul(col, shid, src, start=True, stop=False)
        nc.tensor.matmul(col, bigrow, one11, start=False, stop=True)

    for s in range(LAG):
        _pe_shift(s)

    ct = None
    for s in range(STEPS):
        g = s % G
        if g == 0:
            ct = cstream.tile([PP, G, L + 1], BF16, tag="cg")
            nc.gpsimd.memset(ct[:, :, 0:1], 0.0)
            for k in range(KC):
                lo = s - LAG * k
                cuts = [c for c in (PADF, PADF + PP, PADF + 2 * PP)
                        if lo + PADF < c < lo + PADF + G]
                segs = [lo + PADF] + cuts + [lo + PADF + G]
                for a, bb in zip(segs[:-1], segs[1:]):
                    nc.sync.dma_start(
                        out=ct[k * BPC:(k + 1) * BPC,
                               a - lo - PADF:bb - lo - PADF, 1:L + 1],
                        in_=cost_dram[:, a:bb, k * L:(k + 1) * L])
        for thunk in front_sched.get(s, ()):
            thunk()
        bprev = (s - 1) % 3
        bcur = s % 3
        m1b = s % 2
        nc.vector.tensor_tensor(
            out=M1[:, m1b, 1:L + 1], in0=V[:, bprev, 1:L + 1],
            in1=V[:, bprev, 0:L], op=ALU.min)
        nc.vector.tensor_tensor_scan(
            out=V[:, bcur, 0:L + 1], data0=M1[:, m1b, 0:L + 1],
            data1=ct[:, g, 0:L + 1],
            initial=bnds[s // 2][:, (s % 2):(s % 2) + 1],
            op0=ALU.min, op1=ALU.add)
        if s + LAG < STEPS:
            _pe_shift(s + LAG)

    # final answers: chunk KC-1's last cell, partitions [120, 128)
    nc.sync.dma_start(out=out[:, :],
                      in_=V[PP - BPC:PP, (STEPS - 1) % 3, L:L + 1])


_NC_CACHE = {}


def _build(variant="full", repeats=1, rep_barrier=False):
    key = (variant, repeats, rep_barrier)
    if key in _NC_CACHE:
        return _NC_CACHE[key]
    nc = bacc.Bacc("TRN2", target_bir_lowering=False, debug=False)
    pred = nc.dram_tensor("pred", [BPC, T, D], F32, kind="ExternalInput").ap()
    targ = nc.dram_tensor("targ", [BPC, T, D], F32, kind="ExternalInput").ap()
    out = nc.dram_tensor("out", [BPC, 1], F32, kind="ExternalOutput").ap()
    with ExitStack() as ctx:
        tc = ctx.enter_context(tile.TileContext(nc))
        _kernel_body(ctx, tc, out, pred, targ, variant=variant, repeats=repeats,
                     rep_barrier=rep_barrier)
    nc.finalize()
    _NC_CACHE[key] = nc
    return nc


def kernel(pred, targ):
    pred = np.ascontiguousarray(np.asarray(pred), dtype=np.float32)
    targ = np.ascontiguousarray(np.asarray(targ), dtype=np.float32)
    assert pred.shape == (B, T, D) and targ.shape == (B, T, D)
    nc = _build("ss")
    in_maps = [
        {"pred": pred[c * BPC:(c + 1) * BPC], "targ": targ[c * BPC:(c + 1) * BPC]}
        for c in range(NCORES)
    ]
    res = run_bass_kernel_spmd(nc, in_maps, core_ids=list(range(NCORES)))
    dists = np.concatenate([res.results[c]["out"][:, 0] for c in range(NCORES)])
    return np.asarray(np.mean(dists.astype(np.float32)), dtype=np.float32)
